# revision 3
# baseline (speedup 1.0000x reference)
"""GQA attention (sliding-window, RoPE, QK-norm) Trainium2 Bass kernel.

Sharding: TP over 4 KV heads x DP over batch 2 -> 8 cores.
Core c handles batch b=c//4, kv head kv=c%4 (4 query heads).

Per-core device pipeline (all shapes hardcoded; see spec in repo docs):
  1. QKV proj (f32r): per token-tile psum [t128, 768]; lhsT = xT chunk.
  2. RoPE + L2-norm (norm computed pre-RoPE; RoPE preserves the norm) +
     gamma scale; cast bf16; PE-transpose to QT/KT [D, T] layout.
  3. S^T = K @ Q^T per (head, key-block): psum [k128, <=1152 q]; ACT exp
     (fused 1/sqrt(D) scale) -> bf16; triangle masks multiplied in.
  4. O = sum_kj expS^T.T @ [V|1]: psum [q128, 129]; col 128 = softmax
     denominator; normalize via reciprocal + scalar mul -> f32r.
  5. y^T = woT @ O^T (f32r), PE-transposing O blocks on the fly.
  Output yT [2048, 2048] = y[b]_partial.T; host sums 4 TP shards + transposes.
"""

import math
import os
import sys

import numpy as np

sys.path.insert(0, "/opt/trn_rl_repo")

import concourse.bass as bass  # noqa: E402
import concourse.mybir as mybir  # noqa: E402
import concourse.tile as tile  # noqa: E402
from concourse import bacc  # noqa: E402
from concourse.bass_utils import run_bass_kernel_spmd  # noqa: E402
from concourse.masks import make_identity  # noqa: E402

B, T, DM = 2, 2048, 2048
H, HK, D = 16, 4, 128
G = H // HK            # query heads per kv head = heads per core
EL = G * D             # 512 local head dims
WINDOW = 1024
ROPE_BASE = 10000.0
EPS = 1e-6
TT = T // 128          # 16 token tiles
CC = DM // 128         # 16 contraction chunks
SPAN = WINDOW // 128 + 1  # 9 key blocks per query block

f32 = mybir.dt.float32
f32r = mybir.dt.float32r
bf16 = mybir.dt.bfloat16
AF = mybir.ActivationFunctionType
OP = mybir.AluOpType

INV_SQRT_D = 1.0 / math.sqrt(D)


def _emit_body(nc, tc, dram, sb, rep):
    """Emit one full pipeline (used once; REPEAT>1 only for benchmarking)."""
    xT, yT = dram["xT"], dram["yT"]
    wo_sb = sb["wo"]
    qkt_sb, v_sb, o_sb = sb["qkt"], sb["v"], sb["o"]
    gqk_sb, md_sb, mw_sb = sb["gqk"], sb["md"], sb["mw"]
    ident_bf, ident_r = sb["ident_bf"], sb["ident_r"]
    cosb, sinb = dram["cosb"], dram["sinb"]

    # ---------------- phase 1+2: projections, rope, norm, transpose --------
    with (
        tc.tile_pool(name=f"wall{rep}", bufs=1) as wallpool,
        tc.tile_pool(name=f"x{rep}", bufs=6) as xpool,
        tc.tile_pool(name=f"cos{rep}", bufs=3) as cpool,
        tc.tile_pool(name=f"rope{rep}", bufs=3) as rpool,
        tc.tile_pool(name=f"qn{rep}", bufs=3) as qnpool,
        tc.tile_pool(name=f"norm{rep}", bufs=3) as npool,
        tc.tile_pool(name=f"junk{rep}", bufs=2) as jpool,
        tc.tile_pool(name=f"psum_p{rep}", bufs=2, space="PSUM") as ppool,
        tc.tile_pool(name=f"psum_t{rep}", bufs=2, space="PSUM") as tpool,
    ):
        wall_sb = wallpool.tile([128, CC * 768], f32r)
        for c in range(CC):
            nc.sync.dma_start(wall_sb[:, c * 768:(c + 1) * 768],
                              dram["wAllT"][c * 128:(c + 1) * 128, :])

        for i in range(TT):
            psum_p = ppool.tile([128, 768], f32)
            for c in range(CC):
                xc = xpool.tile([128, 128], f32r)
                nc.sync.dma_start(xc[:], xT[c * 128:(c + 1) * 128, i * 128:(i + 1) * 128])
                nc.tensor.matmul(psum_p[:, 0:512], xc[:], wall_sb[:, c * 768:c * 768 + 512],
                                 start=(c == 0), stop=(c == CC - 1))
                nc.tensor.matmul(psum_p[:, 512:768], xc[:],
                                 wall_sb[:, c * 768 + 512:(c + 1) * 768],
                                 start=(c == 0), stop=(c == CC - 1))

            # sumsq (pre-rope; rope preserves the norm) via ACT square + accum
            nacc = npool.tile([128, 8], f32, tag="nacc")
            for s in range(5):
                sq_junk = jpool.tile([128, 128], f32)
                nc.scalar.activation(sq_junk[:], psum_p[:, s * 128:(s + 1) * 128],
                                     AF.Square, accum_out=nacc[:, s:s + 1])
            nrm = npool.tile([128, 8], f32, tag="nrm")
            nc.scalar.sqrt(nrm[:, 0:5], nacc[:, 0:5])
            nc.vector.tensor_scalar_add(nrm[:, 0:5], nrm[:, 0:5], EPS)
            inv = npool.tile([128, 8], f32, tag="inv")
            nc.vector.reciprocal(inv[:, 0:5], nrm[:, 0:5])

            # rope on Q|K (cols 0:640): rp = psum*cos + rot_half(psum)*sinM
            cos_t = cpool.tile([128, 128], f32, tag="cos")
            nc.sync.dma_start(cos_t[:], cosb[i * 128:(i + 1) * 128, :])
            sin_t = cpool.tile([128, 128], f32, tag="sin")
            nc.sync.dma_start(sin_t[:], sinb[i * 128:(i + 1) * 128, :])

            rp = rpool.tile([128, 640], f32, tag="rp")
            t2 = rpool.tile([128, 640], f32, tag="t2")
            psum_r = psum_p[:].rearrange("p (s d) -> p s d", d=128)
            rp_r = rp[:].rearrange("p (s d) -> p s d", d=128)
            t2_r = t2[:].rearrange("p (s d) -> p s d", d=128)
            for s in range(5):
                nc.vector.tensor_tensor(out=rp_r[:, s, :], in0=psum_r[:, s, :],
                                        in1=cos_t[:], op=OP.mult)
                nc.vector.tensor_tensor(out=t2_r[:, s, 0:64], in0=psum_r[:, s, 64:128],
                                        in1=sin_t[:, 0:64], op=OP.mult)
                nc.vector.tensor_tensor(out=t2_r[:, s, 64:128], in0=psum_r[:, s, 0:64],
                                        in1=sin_t[:, 64:128], op=OP.mult)
            nc.vector.tensor_add(rp[:], rp[:], t2[:])

            # normalize + gamma + cast bf16
            qn = qnpool.tile([128, 640], bf16)
            for s in range(4):
                nc.vector.scalar_tensor_tensor(
                    out=qn[:, s * 128:(s + 1) * 128], in0=rp[:, s * 128:(s + 1) * 128],
                    scalar=inv[:, s:s + 1], in1=gqk_sb[:], op0=OP.mult, op1=OP.mult)
            nc.vector.tensor_scalar_mul(qn[:, 512:640], rp[:, 512:640], inv[:, 4:5])

            # transpose the 5 [128,128] segments into QT/KT [D, T] bf16
            for s in range(5):
                pt = tpool.tile([128, 128], bf16)
                nc.tensor.transpose(pt[:], qn[:, s * 128:(s + 1) * 128], ident_bf[:])
                nc.scalar.copy(qkt_sb[:, s * T + i * 128:s * T + (i + 1) * 128], pt[:])

            # V' tile (col 128 stays 1.0 from the initial memset)
            nc.scalar.copy(v_sb[:, i * 129:i * 129 + 128], psum_p[:, 640:768])

    # ---------------- phase 3+4: attention ---------------------------------
    with (
        tc.tile_pool(name=f"es{rep}", bufs=SPAN + 1) as espool,
        tc.tile_pool(name=f"small{rep}", bufs=3) as smallpool,
        tc.tile_pool(name=f"psum_st{rep}", bufs=2, space="PSUM") as stpool,
        tc.tile_pool(name=f"psum_o{rep}", bufs=2, space="PSUM") as opool,
    ):
        for h in range(G):
            es_tiles = {}
            for kj in range(TT):
                nq = min(kj + SPAN, TT) - kj
                w = nq * 128
                st = stpool.tile([128, 1152], f32)
                off = 0
                while off < w:
                    n = min(512, w - off)
                    nc.tensor.matmul(st[:, off:off + n],
                                     qkt_sb[:, 4 * T + kj * 128:4 * T + (kj + 1) * 128],
                                     qkt_sb[:, h * T + kj * 128 + off:h * T + kj * 128 + off + n],
                                     start=True, stop=True)
                    off += n
                es = espool.tile([128, 1152], bf16)
                nc.scalar.activation(es[:, 0:w], st[:, 0:w], AF.Exp, scale=INV_SQRT_D)
                nc.vector.tensor_tensor(out=es[:, 0:128], in0=es[:, 0:128],
                                        in1=md_sb[:], op=OP.mult)
                if nq == SPAN:
                    nc.vector.tensor_tensor(out=es[:, 1024:1152], in0=es[:, 1024:1152],
                                            in1=mw_sb[:], op=OP.mult)
                es_tiles[kj] = es

                # O for query block qi == kj (all needed kj' <= kj are ready)
                qi = kj
                kjs = list(range(max(0, qi - SPAN + 1), qi + 1))
                po = opool.tile([128, 129], f32)
                for idx, k2 in enumerate(kjs):
                    nc.tensor.matmul(po[:],
                                     es_tiles[k2][:, (qi - k2) * 128:(qi - k2 + 1) * 128],
                                     v_sb[:, k2 * 129:(k2 + 1) * 129],
                                     start=(idx == 0), stop=(idx == len(kjs) - 1))
                inv_o = smallpool.tile([128, 1], f32)
                nc.vector.reciprocal(inv_o[:], po[:, 128:129])
                nc.vector.tensor_scalar_mul(
                    o_sb[:, h * T + qi * 128:h * T + (qi + 1) * 128],
                    po[:, 0:128], inv_o[:])

    # ---------------- phase 5: y^T = woT @ O^T -----------------------------
    with (
        tc.tile_pool(name=f"ot{rep}", bufs=8) as otpool,
        tc.tile_pool(name=f"ystage{rep}", bufs=4) as ypool,
        tc.tile_pool(name=f"psum_t2{rep}", bufs=4, space="PSUM") as tpool2,
        tc.tile_pool(name=f"psum_y{rep}", bufs=3, space="PSUM") as typool,
    ):
        for tc_i in range(4):
            ot_stage = []
            for h in range(G):
                ots = otpool.tile([128, 512], f32r)
                for j in range(4):
                    pt = tpool2.tile([128, 128], f32r)
                    nc.tensor.transpose(
                        pt[:],
                        o_sb[:, h * T + tc_i * 512 + j * 128:h * T + tc_i * 512 + (j + 1) * 128],
                        ident_r[:])
                    nc.vector.tensor_copy(ots[:, j * 128:(j + 1) * 128], pt[:])
                ot_stage.append(ots)
            for dm in range(16):
                py = typool.tile([128, 512], f32)
                for h in range(G):
                    nc.tensor.matmul(py[:], wo_sb[:, h * DM + dm * 128:h * DM + (dm + 1) * 128],
                                     ot_stage[h][:], start=(h == 0), stop=(h == G - 1))
                ys = ypool.tile([128, 512], f32)
                if dm % 2 == 0:
                    nc.scalar.copy(ys[:], py[:])
                else:
                    nc.vector.tensor_copy(ys[:], py[:])
                nc.sync.dma_start(yT[dm * 128:(dm + 1) * 128, tc_i * 512:(tc_i + 1) * 512],
                                  ys[:])


def build_nc(repeat=1):
    nc = bacc.Bacc(None)
    dram = {
        "xT": nc.dram_tensor("xT", [DM, T], f32r, kind="ExternalInput"),
        "wAllT": nc.dram_tensor("wAllT", [DM, EL + 2 * D], f32r, kind="ExternalInput"),
        "woT": nc.dram_tensor("woT", [EL, DM], f32r, kind="ExternalInput"),
        "cosb": nc.dram_tensor("cosb", [T, D], f32, kind="ExternalInput"),
        "sinb": nc.dram_tensor("sinb", [T, D], f32, kind="ExternalInput"),
        "gqk": nc.dram_tensor("gqk", [128, D], f32, kind="ExternalInput"),
        "mdiag": nc.dram_tensor("mdiag", [128, 128], bf16, kind="ExternalInput"),
        "mwin": nc.dram_tensor("mwin", [128, 128], bf16, kind="ExternalInput"),
        "yT": nc.dram_tensor("yT", [DM, T], f32, kind="ExternalOutput"),
    }

    sb = {
        "wo": nc.alloc_sbuf_tensor("wo_sb", [128, G * DM], f32r).ap(),
        "qkt": nc.alloc_sbuf_tensor("qkt_sb", [128, 5 * T], bf16).ap(),
        "v": nc.alloc_sbuf_tensor("v_sb", [128, TT * 129], bf16).ap(),
        "o": nc.alloc_sbuf_tensor("o_sb", [128, G * T], f32r).ap(),
        "gqk": nc.alloc_sbuf_tensor("gqk_sb", [128, D], f32).ap(),
        "md": nc.alloc_sbuf_tensor("md_sb", [128, 128], bf16).ap(),
        "mw": nc.alloc_sbuf_tensor("mw_sb", [128, 128], bf16).ap(),
        "ident_bf": nc.alloc_sbuf_tensor("ident_bf", [128, 128], bf16).ap(),
        "ident_r": nc.alloc_sbuf_tensor("ident_r", [128, 128], f32r).ap(),
        "ident_f": nc.alloc_sbuf_tensor("ident_f", [128, 128], f32).ap(),
    }

    with tile.TileContext(nc) as tc:
        # one-time loads / constants
        for h in range(G):
            nc.sync.dma_start(sb["wo"][:, h * DM:(h + 1) * DM],
                              dram["woT"][h * 128:(h + 1) * 128, :])
        nc.sync.dma_start(sb["gqk"][:], dram["gqk"][:])
        nc.sync.dma_start(sb["md"][:], dram["mdiag"][:])
        nc.sync.dma_start(sb["mw"][:], dram["mwin"][:])
        make_identity(nc, sb["ident_f"])
        nc.vector.tensor_copy(sb["ident_bf"], sb["ident_f"])
        nc.vector.tensor_copy(sb["ident_r"], sb["ident_f"])
        nc.vector.memset(sb["v"], 1.0)

        for rep in range(repeat):
            _emit_body(nc, tc, dram, sb, rep)

    nc.finalize()
    return nc, dram


def host_prep(x, wq, wk, wv, wo, gamma_q, gamma_k):
    """Build per-core input maps (host-side sharding + layout prep)."""
    # rope tables, computed like the reference (float32)
    inv_freq = (1.0 / (ROPE_BASE ** (np.arange(0, D, 2, dtype=np.float32) / D))).astype(np.float32)
    ang = np.arange(T, dtype=np.float32)[:, None] * inv_freq[None, :]
    cosb = np.concatenate([np.cos(ang), np.cos(ang)], axis=-1).astype(np.float32)
    sin = np.sin(ang).astype(np.float32)
    sinb = np.concatenate([-sin, sin], axis=-1).astype(np.float32)

    gqk = np.tile((gamma_q * gamma_k).astype(np.float32)[None, :], (128, 1))

    import ml_dtypes
    k_idx = np.arange(128)[:, None]
    q_idx = np.arange(128)[None, :]
    mdiag = (k_idx <= q_idx).astype(ml_dtypes.bfloat16)
    mwin = (k_idx >= q_idx).astype(ml_dtypes.bfloat16)

    xTs = [np.ascontiguousarray(x[b].T) for b in range(B)]
    in_maps = []
    for core in range(8):
        b, kv = core // 4, core % 4
        wq_s = wq[kv * EL:(kv + 1) * EL, :]
        wk_s = wk[kv * D:(kv + 1) * D, :]
        wv_s = wv[kv * D:(kv + 1) * D, :]
        wAllT = np.ascontiguousarray(np.concatenate([wq_s, wk_s, wv_s], axis=0).T)
        woT = np.ascontiguousarray(wo[:, kv * EL:(kv + 1) * EL].T)
        in_maps.append({
            "xT": xTs[b],
            "wAllT": wAllT.astype(np.float32),
            "woT": woT.astype(np.float32),
            "cosb": cosb,
            "sinb": sinb,
            "gqk": gqk,
            "mdiag": mdiag,
            "mwin": mwin,
        })
    return in_maps


_CACHED = {}


def kernel(x, wq, wk, wv, wo, gamma_q, gamma_k):
    repeat = int(os.environ.get("KERNEL_REPEAT", "1"))
    if repeat not in _CACHED:
        _CACHED[repeat] = build_nc(repeat)
    nc, dram = _CACHED[repeat]

    x = np.asarray(x, dtype=np.float32)
    in_maps = host_prep(x, np.asarray(wq, np.float32), np.asarray(wk, np.float32),
                        np.asarray(wv, np.float32), np.asarray(wo, np.float32),
                        np.asarray(gamma_q, np.float32), np.asarray(gamma_k, np.float32))

    res = run_bass_kernel_spmd(nc, in_maps, core_ids=list(range(8)))
    y = np.zeros((B, T, DM), dtype=np.float32)
    for core in range(8):
        b = core // 4
        y[b] += res.results[core]["yT"].T
    return y
